# revision 1
# baseline (speedup 1.0000x reference)
"""Trainium2 Bass kernel for nn_Attention (dense transformer block-attention).

Reference semantics (faithful reshape WITHOUT head transpose):
  qkv = x @ w_qkv                    # [B, N, 3*1024]
  q = qkv[..., 0:1024].reshape(B, 16, 2048, 64)   # head h <- token rows [h*128,(h+1)*128)
  out[b, n, c] = O_head(n//128)[(n%128)*16 + c//64, c%64]

Sharding: 32 (b, head) pairs over 8 cores -> each core: 1 batch x 4 heads.
Pure data parallel, no collectives. Host preps xT (bf16) per core + full w (bf16).

Layout tricks:
- Sub-token permutation n2' = cb*128 + r (softmax is permutation-invariant
  over keys; queries un-permuted via the output index mapping).
- qT/kT hold the 64-wide head dim DUPLICATED on both partition halves, so
  S matmuls contract K=128 (computing 2*q.k; factor folded into exp scale)
  and the layout transposes are clean [128,128] PE transposes.
- PV: out^T = [v|ones].T @ exp(S^T): softmax denominators ride in row 64.
- One PSUM layout all kernel long: tag ps = 2x[128,1024] (4 banks) used by
  projection accumulators / S ping-pong / tail transposes, tag po =
  1x[65,2048] (4 banks) for PV accumulators. No phase barriers.
"""

import numpy as np
import ml_dtypes

B, N, D = 2, 2048, 1024
H_PER_CORE = 4          # head-blocks per core
ROWS = 128              # token rows per head-block
SUB = 2048              # sub-tokens per head (128 rows * 16 col-blocks)
DH = 64                 # head dim
CB = 16                 # col-blocks per row
SCALE = 0.125           # 64 ** -0.5
N_CORES = 8

_GRAPH = None


def build_graph():
    global _GRAPH
    if _GRAPH is not None:
        return _GRAPH

    import concourse.mybir as mybir
    import concourse.tile as tile
    from concourse import bacc
    from concourse.masks import make_identity
    from contextlib import ExitStack

    f32 = mybir.dt.float32
    bf16 = mybir.dt.bfloat16
    EXP = mybir.ActivationFunctionType.Exp

    nc = bacc.Bacc("TRN2", target_bir_lowering=False, debug=False,
                   num_devices=N_CORES)

    xt_dram = nc.dram_tensor("xt", [D, H_PER_CORE * ROWS], bf16,
                             kind="ExternalInput")
    w_dram = nc.dram_tensor("w", [D, 3 * D], bf16, kind="ExternalInput")
    out_dram = nc.dram_tensor("out", [H_PER_CORE * ROWS, D], f32,
                              kind="ExternalOutput")

    KO = D // 128  # 8 k-tiles

    with tile.TileContext(nc) as tc, ExitStack() as ctx:
        const_pool = ctx.enter_context(tc.tile_pool(name="const", bufs=1))
        in_pool = ctx.enter_context(tc.tile_pool(name="inputs", bufs=1))
        qk_pool = ctx.enter_context(tc.tile_pool(name="qk", bufs=4))
        head_pool = ctx.enter_context(tc.tile_pool(name="head", bufs=1))
        pt_pool = ctx.enter_context(tc.tile_pool(name="pt", bufs=4))
        ot_pool = ctx.enter_context(tc.tile_pool(name="ot", bufs=3))
        small_pool = ctx.enter_context(tc.tile_pool(name="small", bufs=16))
        trt_pool = ctx.enter_context(tc.tile_pool(name="trt", bufs=16))
        psum = ctx.enter_context(tc.tile_pool(name="psum", bufs=2,
                                              space="PSUM"))
        psum2 = ctx.enter_context(tc.tile_pool(name="psum2", bufs=2,
                                               space="PSUM"))
        opsum = ctx.enter_context(tc.tile_pool(name="opsum", bufs=1,
                                               space="PSUM"))

        # ---- constants ----
        ident = const_pool.tile([128, 128], f32, tag="ident")
        make_identity(nc, ident[:])
        ident_bf = const_pool.tile([128, 128], bf16, tag="ident_bf")
        make_identity(nc, ident_bf[:])
        # warm up the exp table while the projection runs
        warm = const_pool.tile([128, 1], f32, tag="warm")
        nc.vector.memset(warm[:], 0.0)
        nc.scalar.activation(warm[:], warm[:], EXP)

        # ---- input DMA in first-consumption order ----
        xt_sbuf = in_pool.tile([128, KO, H_PER_CORE * ROWS], bf16, tag="xt")
        w_sbuf = in_pool.tile([128, KO, 3 * D], bf16, tag="w")
        nc.sync.dma_start(xt_sbuf[:, 0, 0:ROWS],
                          xt_dram.ap()[0:128, 0:ROWS])
        nc.sync.dma_start(w_sbuf[:, 0, 0:512], w_dram.ap()[0:128, 0:512])
        nc.sync.dma_start(xt_sbuf[:, 0, ROWS:],
                          xt_dram.ap()[0:128, ROWS:])
        nc.sync.dma_start(w_sbuf[:, 0, 512:1024],
                          w_dram.ap()[0:128, 512:1024])
        for ko in range(1, KO):
            nc.sync.dma_start(xt_sbuf[:, ko, :],
                              xt_dram.ap()[ko * 128:(ko + 1) * 128, :])
            nc.sync.dma_start(
                w_sbuf[:, ko, 0:1024],
                w_dram.ap()[ko * 128:(ko + 1) * 128, 0:1024])
        for half in range(1, 3):
            for ko in range(KO):
                nc.sync.dma_start(
                    w_sbuf[:, ko, half * 1024:(half + 1) * 1024],
                    w_dram.ap()[ko * 128:(ko + 1) * 128,
                                half * 1024:(half + 1) * 1024])

        # persistent per-head tiles (qT/kT carry duplicated d-halves)
        qT = [head_pool.tile([128, SUB], bf16, tag=f"qT{t}", name=f"qT{t}")
              for t in range(H_PER_CORE)]
        kT = [head_pool.tile([128, SUB], bf16, tag=f"kT{t}", name=f"kT{t}")
              for t in range(H_PER_CORE)]
        v_ones = [head_pool.tile([128, CB, DH + 1], bf16, tag=f"vo{t}",
                                 name=f"vo{t}")
                  for t in range(H_PER_CORE)]
        for t in range(H_PER_CORE):
            nc.vector.memset(v_ones[t][:, :, DH], 1.0)

        # ---- phase 1: projection (per block) ----
        qk2s = [None] * H_PER_CORE

        def emit_proj(t):
            qk2 = qk_pool.tile([128, 2 * CB, 128], bf16, tag="qk2",
                               name=f"qk2_{t}")
            qk2s[t] = qk2
            # six [128,512] accumulators from the small psum pool so the
            # attention pipeline (ps ping-pong + po) is never contended
            for ncx in range(6):
                ps = psum2.tile([128, 512], f32, tag="ps2")
                for ko in range(KO):
                    nc.tensor.matmul(
                        ps[:],
                        xt_sbuf[:, ko, t * ROWS:(t + 1) * ROWS],
                        w_sbuf[:, ko, ncx * 512:(ncx + 1) * 512],
                        start=(ko == 0), stop=(ko == KO - 1))
                src = ps[:].rearrange("p (a b) -> p a b", b=DH)
                if ncx < 4:
                    nc.vector.tensor_copy(
                        qk2[:, ncx * 8:(ncx + 1) * 8, 0:DH], src)
                    nc.vector.tensor_copy(
                        qk2[:, ncx * 8:(ncx + 1) * 8, DH:128], src)
                else:
                    nc.vector.tensor_copy(
                        v_ones[t][:, (ncx - 4) * 8:(ncx - 3) * 8, 0:DH],
                        src)

        def emit_transposes(t):
            if False:
                # XBAR DMA-transpose on the Sync engine: runs ahead of
                # need, overlapping earlier attention on PE/ACT.
                for cb in range(2 * CB):
                    dst = qT[t] if cb < CB else kT[t]
                    nc.sync.dma_start_transpose(
                        dst[:, (cb % CB) * 128:((cb % CB) + 1) * 128],
                        qk2s[t][:, cb, :])
            else:
                # head 0 gates the first attention: PE transposes (fast)
                for cb in range(2 * CB):
                    pst = psum2.tile([128, 128], bf16, tag="ps2")
                    nc.tensor.transpose(pst[:], qk2s[t][:, cb, :],
                                        ident_bf[:])
                    dst = qT[t] if cb < CB else kT[t]
                    nc.vector.tensor_copy(
                        dst[:, (cb % CB) * 128:((cb % CB) + 1) * 128],
                        pst[:])

        def emit_pass(t, ihalf, OTt):
            po = opsum.tile([DH + 1, SUB // 2], f32, tag="po")
            for j in range(CB):
                ps = psum.tile([128, 1024], f32, tag="ps")
                for sub in range(2):
                    ic = ihalf * 2 + sub
                    nc.tensor.matmul(
                        ps[:, sub * 512:(sub + 1) * 512],
                        kT[t][:, j * 128:(j + 1) * 128],
                        qT[t][:, ic * 512:(ic + 1) * 512],
                        start=True, stop=True)
                pt = pt_pool.tile([128, 1024], bf16, tag="pt")
                # psum holds 2*(q.k) (duplicated halves) -> scale/2
                nc.scalar.activation(pt[:], ps[:], EXP, scale=SCALE / 2)
                for sub in range(2):
                    nc.tensor.matmul(
                        po[:, sub * 512:(sub + 1) * 512],
                        v_ones[t][:, j, :],
                        pt[:, sub * 512:(sub + 1) * 512],
                        start=(j == 0), stop=(j == CB - 1))
            # evacuate the half-accumulator on DVE (idle during attention;
            # first in its FIFO region so the po slot frees promptly)
            nc.vector.tensor_copy(
                OTt[0:DH + 1, ihalf * 1024:(ihalf + 1) * 1024], po[:])

        def emit_tail_half(t, ihalf, OTt):
            # PE transpose (psum2) -> normalize (DVE) -> out DMA.  The
            # final head's outs go on the (empty) Sync HWDGE queue so the
            # kernel end isn't paced by GpSimd's slow SWDGE issue.
            out_eng = nc.sync if t == H_PER_CORE - 1 else nc.gpsimd
            for cb in range(ihalf * 8, ihalf * 8 + 8):
                ptr = psum2.tile([128, DH + 1], bf16, tag="ps2")
                nc.tensor.transpose(
                    ptr[:],
                    OTt[0:DH + 1, cb * 128:(cb + 1) * 128],
                    ident_bf[0:DH + 1, 0:DH + 1])
                recip = small_pool.tile([128, 1], f32, tag="recip")
                nc.vector.reciprocal(recip[:], ptr[:, DH:DH + 1])
                outt = small_pool.tile([128, DH], f32, tag="outt")
                nc.vector.tensor_scalar_mul(outt[:], ptr[:, 0:DH],
                                            recip[:])
                out_eng.dma_start(
                    out_dram.ap()[t * ROWS:(t + 1) * ROWS,
                                  cb * DH:(cb + 1) * DH],
                    outt[:])

        # ---- program order: later heads' projection/transposes deferred
        # so they fill PE gaps inside earlier ACT-bound attention ----
        OTs = [ot_pool.tile([DH + 1, SUB], bf16, tag="OTf", name=f"OTf{t}")
               for t in range(H_PER_CORE)]
        emit_proj(0)
        emit_transposes(0)
        emit_pass(0, 0, OTs[0])
        emit_proj(1)
        emit_transposes(1)
        emit_pass(0, 1, OTs[0])
        emit_tail_half(0, 0, OTs[0])
        emit_proj(2)
        emit_transposes(2)
        emit_pass(1, 0, OTs[1])
        emit_tail_half(0, 1, OTs[0])
        emit_proj(3)
        emit_transposes(3)
        emit_pass(1, 1, OTs[1])
        emit_tail_half(1, 0, OTs[1])
        emit_pass(2, 0, OTs[2])
        emit_tail_half(1, 1, OTs[1])
        emit_pass(2, 1, OTs[2])
        emit_tail_half(2, 0, OTs[2])
        emit_pass(3, 0, OTs[3])
        emit_tail_half(2, 1, OTs[2])
        emit_pass(3, 1, OTs[3])
        emit_tail_half(3, 0, OTs[3])
        emit_tail_half(3, 1, OTs[3])

    nc.compile()
    _GRAPH = nc
    return nc


def make_in_maps(x, w_qkv):
    w_bf = np.ascontiguousarray(w_qkv).astype(ml_dtypes.bfloat16)
    maps = []
    for c in range(N_CORES):
        b = c // 4
        r0 = (c % 4) * H_PER_CORE * ROWS
        xt = np.ascontiguousarray(
            x[b, r0:r0 + H_PER_CORE * ROWS, :].T).astype(ml_dtypes.bfloat16)
        maps.append({"xt": xt, "w": w_bf})
    return maps


def assemble_out(results):
    out = np.empty((B, N, D), dtype=np.float32)
    for c in range(N_CORES):
        b = c // 4
        r0 = (c % 4) * H_PER_CORE * ROWS
        out[b, r0:r0 + H_PER_CORE * ROWS, :] = results[c]["out"]
    return out


def kernel(x, w_qkv):
    from concourse import bass_utils
    nc = build_graph()
    res = bass_utils.run_bass_kernel_spmd(
        nc, make_in_maps(np.asarray(x), np.asarray(w_qkv)),
        list(range(N_CORES)))
    return assemble_out(res.results)



# revision 3
# speedup vs baseline: 1.0935x; 1.0935x over previous
"""Trainium2 Bass kernel for nn_Attention (dense transformer block-attention).

Reference semantics (faithful reshape WITHOUT head transpose):
  qkv = x @ w_qkv                    # [B, N, 3*1024]
  head h <- token rows [h*128,(h+1)*128); pseudo-token n = r*16 + cb,
  q_head[n, dd] = q[h*128 + r, cb*64 + dd]   (cb in [0,16), dd in [0,64))

Sharding: 32 (b, head) pairs over 8 cores -> each core: 1 batch x 4 heads.
Pure data parallel, no collectives. Host preps xT (bf16) per core + full w
(bf16), and performs the final softmax normalization (divide by the
denominator row that rides along the PV accumulation).

Key structure (v2):
- q/k projection computed TRANSPOSED (w stationary, xT moving), so qT/kT
  come out of PSUM directly in [d, token] layout: no PE transposes at all.
  A 128-col w c-tile covers col-blocks (2m, 2m+1): psum parts 0-63 hold
  d of even cb, parts 64-127 hold d of odd cb.
- S matmuls contract K=64 (true head dim) as row-tiled concurrent PAIRS:
  tile (0,0) does even-cb keys x even-cb queries, tile (64,0) does odd-cb
  keys x odd-cb queries.  Cross-parity products use kTs, a partition-
  half-swapped copy of kT made with SBUF->SBUF DMA.
- exp on ACT in [128,1024] chunks (ping-pong PSUM groups), output bf16.
- PV: out^T = [v|ones].T @ exp(S^T): softmax denominators ride in row 64.
  Unnormalized [65, 512] accumulators are DMA'd out; host divides.
- q/k projection is split into head-pair halves (N=256) so head 0's
  prerequisites are small; remaining projection fills PE gaps under the
  ACT-bound attention stream.
"""

import numpy as np
import ml_dtypes

B, N, D = 2, 2048, 1024
H_PER_CORE = 4
ROWS = 128
DH = 64
CB = 16
SCALE = 0.125            # 64 ** -0.5
N_CORES = 8
KO = D // 128            # 8 k-tiles

_GRAPH = None


def build_graph():
    global _GRAPH
    if _GRAPH is not None:
        return _GRAPH

    import concourse.mybir as mybir
    import concourse.tile as tile
    from concourse import bacc
    from contextlib import ExitStack

    f32 = mybir.dt.float32
    bf16 = mybir.dt.bfloat16
    EXP = mybir.ActivationFunctionType.Exp

    nc = bacc.Bacc("TRN2", target_bir_lowering=False, debug=False,
                   num_devices=N_CORES)

    xt_dram = nc.dram_tensor("xt", [D, H_PER_CORE * ROWS], bf16,
                             kind="ExternalInput")
    w_dram = nc.dram_tensor("w", [D, 3 * D], bf16, kind="ExternalInput")
    out_dram = nc.dram_tensor("out", [DH + 1, 8192], f32,
                              kind="ExternalOutput")

    with tile.TileContext(nc) as tc, ExitStack() as ctx:
        const_pool = ctx.enter_context(tc.tile_pool(name="const", bufs=1))
        in_pool = ctx.enter_context(tc.tile_pool(name="inputs", bufs=1))
        head_pool = ctx.enter_context(tc.tile_pool(name="head", bufs=1))
        pt_pool = ctx.enter_context(tc.tile_pool(name="pt", bufs=4))
        ot_pool = ctx.enter_context(tc.tile_pool(name="ot", bufs=4))
        psA = ctx.enter_context(tc.tile_pool(name="psA", bufs=2,
                                             space="PSUM"))
        psPO = ctx.enter_context(tc.tile_pool(name="psPO", bufs=1,
                                              space="PSUM"))
        psPJ = ctx.enter_context(tc.tile_pool(name="psPJ", bufs=2,
                                              space="PSUM"))

        # warm up the exp table before the attention stream needs it
        warm = const_pool.tile([128, 1], f32, tag="warm")
        nc.vector.memset(warm[:], 0.0)
        nc.scalar.activation(warm[:], warm[:], EXP)

        # ---- input DMA in first-consumption order ----
        xt_sbuf = in_pool.tile([128, KO, H_PER_CORE * ROWS], bf16, tag="xt")
        w_sbuf = in_pool.tile([128, KO, 3 * D], bf16, tag="w")
        # h01 rows of xT first (all we need to start)
        for ko in range(KO):
            nc.sync.dma_start(xt_sbuf[:, ko, 0:256],
                              xt_dram.ap()[ko * 128:(ko + 1) * 128, 0:256])
        # w: k c-tile 0 (cols 1024..1152), then q c-tiles 0-3 (cols 0..512)
        for ko in range(KO):
            nc.sync.dma_start(
                w_sbuf[:, ko, 1024:1152],
                w_dram.ap()[ko * 128:(ko + 1) * 128, 1024:1152])
        for ko in range(KO):
            nc.sync.dma_start(w_sbuf[:, ko, 0:512],
                              w_dram.ap()[ko * 128:(ko + 1) * 128, 0:512])
        # v cols (needed by v0 projection before the first PV)
        for ko in range(KO):
            nc.sync.dma_start(
                w_sbuf[:, ko, 2048:3072],
                w_dram.ap()[ko * 128:(ko + 1) * 128, 2048:3072])
        # remaining k c-tiles 1-7
        for ko in range(KO):
            nc.sync.dma_start(
                w_sbuf[:, ko, 1152:2048],
                w_dram.ap()[ko * 128:(ko + 1) * 128, 1152:2048])
        # remaining q c-tiles 4-7
        for ko in range(KO):
            nc.sync.dma_start(w_sbuf[:, ko, 512:1024],
                              w_dram.ap()[ko * 128:(ko + 1) * 128, 512:1024])
        # h23 rows of xT
        for ko in range(KO):
            nc.sync.dma_start(xt_sbuf[:, ko, 256:512],
                              xt_dram.ap()[ko * 128:(ko + 1) * 128, 256:512])

        # persistent tiles: [part, m, t, r]; parts 0-63 = d of cb 2m,
        # parts 64-127 = d of cb 2m+1 (kT) / queries likewise (qT).
        qT = head_pool.tile([128, 8, 4, 128], bf16, tag="qT", name="qT")
        kT = head_pool.tile([128, 8, 4, 128], bf16, tag="kT", name="kT")
        kTs = head_pool.tile([128, 8, 4, 128], bf16, tag="kTs", name="kTs")
        v_ones = [head_pool.tile([128, CB, DH + 1], bf16, tag=f"vo{t}",
                                 name=f"vo{t}")
                  for t in range(H_PER_CORE)]
        for t in range(H_PER_CORE):
            nc.vector.memset(v_ones[t][:, :, DH], 1.0)

        # ---- projection units ----
        def proj_qk(sec, m, hp):
            # sec: 0 = q, 1 = k; m: c-tile (cb pair (2m, 2m+1));
            # hp: 0 = heads 0-1 (rows 0:256), 1 = heads 2-3 (rows 256:512)
            ps = psPJ.tile([128, 256], f32, tag="pj")
            c0 = sec * 1024 + m * 128
            for ko in range(KO):
                nc.tensor.matmul(
                    ps[:],
                    w_sbuf[:, ko, c0:c0 + 128],
                    xt_sbuf[:, ko, hp * 256:(hp + 1) * 256],
                    start=(ko == 0), stop=(ko == KO - 1))
            dst = qT if sec == 0 else kT
            nc.vector.tensor_copy(dst[:, m, hp * 2:(hp + 1) * 2, :], ps[:])
            if sec == 1:
                # partition-half swap for cross-parity S matmuls
                nc.sync.dma_start(kTs[64:128, m, hp * 2:(hp + 1) * 2, :],
                                  kT[0:64, m, hp * 2:(hp + 1) * 2, :])
                nc.sync.dma_start(kTs[0:64, m, hp * 2:(hp + 1) * 2, :],
                                  kT[64:128, m, hp * 2:(hp + 1) * 2, :])

        def proj_v(t, half):
            ps = psPJ.tile([128, 512], f32, tag="pj")
            c0 = 2048 + half * 512
            for ko in range(KO):
                nc.tensor.matmul(
                    ps[:],
                    xt_sbuf[:, ko, t * 128:(t + 1) * 128],
                    w_sbuf[:, ko, c0:c0 + 512],
                    start=(ko == 0), stop=(ko == KO - 1))
            src = ps[:].rearrange("p (a b) -> p a b", b=DH)
            nc.vector.tensor_copy(
                v_ones[t][:, half * 8:(half + 1) * 8, 0:DH], src)

        # ---- attention step ----
        def step(t, m, s, cross, po_e, po_o, first, last):
            kt = kTs if cross else kT
            ps = psA.tile([128, 1024], f32, tag="psA")
            nc.tensor.matmul(ps[:, 0:512],
                             kt[0:64, m, t, :],
                             qT[0:64, s * 4:(s + 1) * 4, t, :],
                             start=True, stop=True)
            nc.tensor.matmul(ps[:, 512:1024],
                             kt[64:128, m, t, :],
                             qT[64:128, s * 4:(s + 1) * 4, t, :],
                             start=True, stop=True)
            pt = pt_pool.tile([128, 1024], bf16, tag="pt")
            nc.scalar.activation(pt[:], ps[:], EXP, scale=SCALE)
            cb_lo = 2 * m + (1 if cross else 0)
            cb_hi = 2 * m + (0 if cross else 1)
            nc.tensor.matmul(po_e[:], v_ones[t][:, cb_lo, :],
                             pt[:, 0:512], start=first, stop=last)
            nc.tensor.matmul(po_o[:], v_ones[t][:, cb_hi, :],
                             pt[:, 512:1024], start=first, stop=last)

        def evac(t, s, po_e, po_o):
            for p, po in ((0, po_e), (1, po_o)):
                ot = ot_pool.tile([DH + 1, 512], f32, tag="ot")
                nc.vector.tensor_copy(ot[:], po[:])
                base = ((t * 2 + p) * 2 + s) * 512
                eng = nc.sync if t == H_PER_CORE - 1 else nc.gpsimd
                eng.dma_start(out_dram.ap()[0:DH + 1, base:base + 256],
                              ot[:, 0:256])
                eng.dma_start(out_dram.ap()[0:DH + 1, base + 256:base + 512],
                              ot[:, 256:512])

        # ---- filler schedule: proj units woven between attention steps.
        # Key ordering constraints:
        #   kT h01 c-tile m+1 needed at straight-step m+1 of heads 0/1
        #   qT h01 c-tiles 4-7 needed at s=1 of heads 0/1
        #   v_t needed at first PV of head t
        #   h23 tiles needed from head 2 on
        fillers = []
        fillers += [("k", mm, 0) for mm in range(1, 8)]      # 7 units
        fillers += [("q", mm, 0) for mm in range(4, 8)]      # 4
        fillers += [("v", 1, 0), ("v", 1, 1)]                # 2
        fillers += [("k", mm, 1) for mm in range(0, 8)]      # 8
        fillers += [("q", mm, 1) for mm in range(0, 8)]      # 8
        fillers += [("v", 2, 0), ("v", 2, 1)]                # 2
        fillers += [("v", 3, 0), ("v", 3, 1)]                # 2
        fillers.reverse()  # pop() from the end

        def emit_filler():
            if fillers:
                kind, a, b = fillers.pop()
                if kind == "v":
                    proj_v(a, b)
                else:
                    proj_qk(0 if kind == "q" else 1, a, b)

        # ---- startup projection (minimum prerequisites for head 0) ----
        proj_qk(1, 0, 0)                 # kT c-tile 0, heads 0-1
        for mm in range(4):
            proj_qk(0, mm, 0)            # qT c-tiles 0-3, heads 0-1
        proj_v(0, 0)
        proj_v(0, 1)

        # ---- main attention loop ----
        # filler pacing: one unit after every step where available, but
        # hold back so early steps are not starved (kT tile m+1 emitted
        # right after straight step m of head 0).
        for t in range(H_PER_CORE):
            for s in range(2):
                po_e = psPO.tile([DH + 1, 512], f32, tag="poe")
                po_o = psPO.tile([DH + 1, 512], f32, tag="poo")
                for cross in range(2):
                    for m in range(8):
                        step(t, m, s, cross, po_e, po_o,
                             first=(cross == 0 and m == 0),
                             last=(cross == 1 and m == 7))
                        emit_filler()
                evac(t, s, po_e, po_o)

    nc.compile()
    _GRAPH = nc
    return nc


def make_in_maps(x, w_qkv):
    w_bf = np.ascontiguousarray(w_qkv).astype(ml_dtypes.bfloat16)
    maps = []
    for c in range(N_CORES):
        b = c // 4
        r0 = (c % 4) * H_PER_CORE * ROWS
        xt = np.ascontiguousarray(
            x[b, r0:r0 + H_PER_CORE * ROWS, :].T).astype(ml_dtypes.bfloat16)
        maps.append({"xt": xt, "w": w_bf})
    return maps


def assemble_out(results):
    out = np.empty((B, N, D), dtype=np.float32)
    for c in range(N_CORES):
        b, quad = divmod(c, 4)
        arr = np.asarray(results[c]["out"], dtype=np.float32)
        arr = arr.reshape(DH + 1, 4, 2, 2, 4, 128)   # [part,t,p,s,mi,r]
        num = arr[0:DH]
        den = arr[DH]
        ratio = num / den[None]                      # [dd,t,p,s,mi,r]
        tmp = ratio.transpose(1, 5, 3, 4, 2, 0)      # [t,r,s,mi,p,dd]
        out[b, quad * 512:(quad + 1) * 512, :] = tmp.reshape(512, 1024)
    return out


def kernel(x, w_qkv):
    from concourse import bass_utils
    nc = build_graph()
    res = bass_utils.run_bass_kernel_spmd(
        nc, make_in_maps(np.asarray(x), np.asarray(w_qkv)),
        list(range(N_CORES)))
    return assemble_out(res.results)
